# revision 69
# baseline (speedup 1.0000x reference)
import sys

for _p in ("/opt/trn_rl_repo",):
    if _p not in sys.path:
        sys.path.append(_p)

"""AttnBlock (GroupNorm + single-head self-attention + residual) Bass/Tile
kernel for one NeuronCore (one batch sample), channel-major layout.

Per-core problem:  x [C=512, HW] f32
  hn = groupnorm(x, 32 groups, eps=1e-5) * gn_w + gn_b
  q/k/v = 1x1 conv (C x C) on tokens;  scores = (q k^T) / sqrt(C)
  attn = softmax(scores);  o = attn @ v;  out = x + (o @ wo^T + bo)

Layout strategy (all big matmuls in bf16: same 1 col/cycle PE rate as
f32r but enables fast-weight-load so LDWEIGHTS hides behind streaming;
bf16 also halves SBUF so Qt/Kt/V and x all stay resident -> single-pass
phase B with zero DRAM spills):
  - x kept in SBUF as bf16 after the stats pass (residual + pass-2 reads)
  - hn, Qt, Kt channel-major [c, hw];  V token-major [hw, c]
  - scores computed transposed St[j, q] = sum_c Kt[c,j] Qt[c,q]
  - exp via ACT, no max subtraction (scores ~N(0,1) by construction)
  - softmax denominator: elementwise accumulate exp tiles on DVE, then a
    ones-vector matmul for the partition sum; normalization applied to
    O^T after the PV accumulation (rank-1 ones matmul broadcasts 1/d)
  - PV: O^T[c, q] += V[j, :]^T P^T[j, q] accumulated in PSUM over all
    32 j-tiles (single pass)
  - final wo projection + bias + residual of q-block N deferred into
    q-block N+1's PE stream to fill the scores->exp->PV latency bubble
"""

from contextlib import ExitStack

import concourse.bass as bass
import concourse.tile as tile
from concourse import mybir
from concourse.masks import make_identity

F32 = mybir.dt.float32
F32R = mybir.dt.float32r
BF16 = mybir.dt.bfloat16
FP8 = mybir.dt.float8e4
DR = mybir.MatmulPerfMode.DoubleRow
AX = mybir.AxisListType
OP = mybir.AluOpType
ACTF = mybir.ActivationFunctionType
# softmax exp shift: cancels in normalization, keeps fp8 exp values in the
# healthy e4m3 range (robust to denormal flush either way; see sims)
EXP_BIAS = -2.0

C = 512
NCH = 4  # channel chunks of 128
GPC = 8  # groups per 128-channel chunk (16 channels per group)
EPS = 1e-5


def build(nc: bass.Bass, HW: int = 4096):
    SCALE_Q = float(C) ** (-0.5)
    NJB = HW // 512      # 512-col j/q blocks
    NQB = HW // 512
    NJT = HW // 128      # 128-col j tiles
    GN_N = 16 * HW       # elements per group

    x = nc.dram_tensor("x", [C, HW], F32, kind="ExternalInput")
    gn_w = nc.dram_tensor("gn_w", [C], F32, kind="ExternalInput")
    gn_b = nc.dram_tensor("gn_b", [C], F32, kind="ExternalInput")
    wq = nc.dram_tensor("wq", [C, C], F32, kind="ExternalInput")
    bq = nc.dram_tensor("bq", [C], F32, kind="ExternalInput")
    wk = nc.dram_tensor("wk", [C, C], F32, kind="ExternalInput")
    bk = nc.dram_tensor("bk", [C], F32, kind="ExternalInput")
    wv = nc.dram_tensor("wv", [C, C], F32, kind="ExternalInput")
    bv = nc.dram_tensor("bv", [C], F32, kind="ExternalInput")
    wo = nc.dram_tensor("wo", [C, C], F32, kind="ExternalInput")
    bo = nc.dram_tensor("bo", [C], F32, kind="ExternalInput")
    out = nc.dram_tensor("out", [C, HW], F32, kind="ExternalOutput")

    x_r = x.rearrange("(c p) q -> p c q", p=128)
    out_r = out.rearrange("(c p) q -> p c q", p=128)

    with tile.TileContext(nc) as tc, ExitStack() as ctx:
        pconst = ctx.enter_context(tc.tile_pool(name="const", bufs=1))
        ppersist = ctx.enter_context(tc.tile_pool(name="persist", bufs=1))

        # ---- constants ----
        identity = pconst.tile([128, 128], F32, tag="ident")
        make_identity(nc, identity[:])
        ones128_f = pconst.tile([128, 1], F32, tag="ones128_f")
        nc.gpsimd.memset(ones128_f[:], 1.0)
        ones128 = pconst.tile([128, 1], F32R, tag="ones128")
        nc.vector.tensor_copy(ones128[:], ones128_f[:])
        ones1_f = pconst.tile([1, 128], F32, tag="ones1_f")
        nc.gpsimd.memset(ones1_f[:], 1.0)
        ones1 = pconst.tile([1, 128], F32R, tag="ones1")
        nc.vector.tensor_copy(ones1[:], ones1_f[:])
        # group indicator matrices: ind8[c, g] = e8[g, c] = (c // 16 == g)
        ind8_f = pconst.tile([128, GPC], F32, tag="ind8_f")
        nc.gpsimd.memset(ind8_f[:], 1.0)
        nc.gpsimd.affine_select(
            out=ind8_f[:], in_=ind8_f[:], compare_op=OP.is_ge, fill=0.0,
            base=0, channel_multiplier=1, pattern=[[-16, GPC]],
        )
        nc.gpsimd.affine_select(
            out=ind8_f[:], in_=ind8_f[:], compare_op=OP.is_ge, fill=0.0,
            base=15, channel_multiplier=-1, pattern=[[16, GPC]],
        )
        ind8 = pconst.tile([128, GPC], F32R, tag="ind8")
        nc.vector.tensor_copy(ind8[:], ind8_f[:])
        e8_f = pconst.tile([GPC, 128], F32, tag="e8_f")
        nc.gpsimd.memset(e8_f[:], 1.0)
        nc.gpsimd.affine_select(
            out=e8_f[:], in_=e8_f[:], compare_op=OP.is_ge, fill=0.0,
            base=0, channel_multiplier=-16, pattern=[[1, 128]],
        )
        nc.gpsimd.affine_select(
            out=e8_f[:], in_=e8_f[:], compare_op=OP.is_ge, fill=0.0,
            base=15, channel_multiplier=16, pattern=[[-1, 128]],
        )
        e8 = pconst.tile([GPC, 128], F32R, tag="e8")
        nc.vector.tensor_copy(e8[:], e8_f[:])

        gnw4 = pconst.tile([128, NCH], F32, tag="gnw4")
        gnb4 = pconst.tile([128, NCH], F32, tag="gnb4")
        bq4 = pconst.tile([128, NCH], F32, tag="bq4")
        bk4 = pconst.tile([128, NCH], F32, tag="bk4")
        bo4 = pconst.tile([128, NCH], F32, tag="bo4")
        for t, src in ((gnw4, gn_w), (gnb4, gn_b), (bq4, bq), (bk4, bk), (bo4, bo)):
            nc.sync.dma_start(out=t[:], in_=src.rearrange("(c p) -> p c", p=128))
        bv_row = pconst.tile([1, C], F32, tag="bv_row")
        nc.sync.dma_start(out=bv_row[:], in_=bv.rearrange("(a i) -> a i", a=1))
        bv_row_r = pconst.tile([1, C], F32R, tag="bv_row_r")
        nc.vector.tensor_copy(bv_row_r[:], bv_row[:])
        bv_bcast = pconst.tile([128, C], F32, tag="bv_bcast")

        eps_t = pconst.tile([GPC, 1], F32, tag="eps_t")
        nc.gpsimd.memset(eps_t[:], EPS)
        expb_t = pconst.tile([128, 1], F32, tag="expb_t")
        nc.gpsimd.memset(expb_t[:], EXP_BIAS)
        bnst = pconst.tile([128, NCH, NJB, 6], F32, tag="bnst")
        cmv = pconst.tile([128, NCH, 2], F32, tag="cmv")
        ch_stats_r = pconst.tile([128, NCH, 2], F32R, tag="ch_stats_r")
        scale4 = pconst.tile([128, NCH], F32, tag="scale4")
        shift4 = pconst.tile([128, NCH], F32, tag="shift4")

        # ---- persistent tensors (all resident, no spills) ----
        qt = ppersist.tile([128, NCH, HW], FP8, tag="qt")
        kt = ppersist.tile([128, NCH, HW], FP8, tag="kt")
        vt = ppersist.tile([128, NJT, C], FP8, tag="vt")
        woT = ppersist.tile([128, NCH, C], FP8, tag="woT")
        # raw f32 x stays resident: read from HBM exactly once (stats pass),
        # then the projection pass and the residual consume it from SBUF
        x32 = ppersist.tile([128, NCH, HW], F32, tag="x32")

        # ---- phase A ----
        with tc.tile_pool(name="wqkv", bufs=1) as pwqkv:
            # q/k/v weights upscaled x16 into fp8 (keeps the N(0,1/sqrt(C))
            # entries out of the e4m3 denormal range); the 1/16 is folded
            # into the projection drains
            wqT = pwqkv.tile([128, NCH, C], FP8, tag="wqT")
            wkT = pwqkv.tile([128, NCH, C], FP8, tag="wkT")
            wvT = pwqkv.tile([128, NCH, C], FP8, tag="wvT")

            with tc.tile_pool(name="psA", bufs=1, space="PSUM") as psA:
                with tc.tile_pool(name="scrA", bufs=2) as pscr, \
                     tc.tile_pool(name="raw", bufs=4) as praw:
                    # weight DMAs issued first so transposes can fill the
                    # stats pass on the PE
                    # ---- pass 1: GN statistics (bn_stats one-pass mean/var
                    # on DVE; ACT queue stays free for the transpose drains).
                    # Weight DMAs are interleaved after the first x blocks so
                    # the x stream keeps DMA priority but transposes can still
                    # start early.
                    raws = []
                    wlist = [wq, wk, wv, wo]
                    for jb in range(NJB):
                        nc.sync.dma_start(
                            out=x32[:, :, 512 * jb : 512 * (jb + 1)],
                            in_=x_r[:, :, 512 * jb : 512 * (jb + 1)],
                        )
                        if jb < len(wlist):
                            raw = praw.tile([128, NCH, C], F32, tag="raw")
                            w_r = wlist[jb].rearrange("(c p) i -> p c i", p=128)
                            for co in range(NCH):
                                nc.sync.dma_start(
                                    out=raw[:, co, :], in_=w_r[:, co, :]
                                )
                            raws.append(raw)
                        for ci in range(NCH):
                            nc.vector.bn_stats(
                                bnst[:, ci, jb, :],
                                x32[:, ci, 512 * jb : 512 * (jb + 1)],
                            )
                    # weight transposes: wT[:, ci, co*128:..] = W[co blk, ci blk].T
                    if True:
                        for raw, wT, wscale in zip(
                            raws, (wqT, wkT, wvT, woT), (16.0, 16.0, 16.0, 16.0)
                        ):
                            for co in range(NCH):
                                for ci in range(NCH):
                                    ps = psA.tile([128, 128], F32, tag="m", bufs=4)
                                    nc.tensor.transpose(
                                        ps[:],
                                        raw[:, co, 128 * ci : 128 * (ci + 1)],
                                        identity[:],
                                    )
                                    nc.scalar.activation(
                                        wT[:, ci, 128 * co : 128 * (co + 1)],
                                        ps[:],
                                        ACTF.Identity,
                                        scale=wscale,
                                    )
                        # bv broadcast tile (rank-1 matmul)
                        psbv = psA.tile([128, C], F32, tag="m", bufs=4)
                        nc.tensor.matmul(
                            psbv[:], ones1[:], bv_row_r[:], start=True, stop=True
                        )
                        nc.scalar.activation(bv_bcast[:], psbv[:], ACTF.Identity)
                    # combine stats -> per-channel scale/shift (batched over ci
                    # to minimize cross-engine dependency hops)
                    for ci in range(NCH):
                        nc.vector.bn_aggr(cmv[:, ci, :], bnst[:, ci, :, :])
                    m2 = pscr.tile([128, NCH, 1], F32, tag="st_m2")
                    nc.vector.tensor_mul(m2[:], cmv[:, :, 0:1], cmv[:, :, 0:1])
                    nc.vector.tensor_copy(ch_stats_r[:, :, 0:1], cmv[:, :, 0:1])
                    nc.vector.tensor_add(
                        ch_stats_r[:, :, 1:2], cmv[:, :, 1:2], m2[:]
                    )
                    psg = psA.tile([GPC, NCH, 2], F32, tag="t", bufs=2)
                    nc.tensor.matmul(
                        psg[:], ind8[:], ch_stats_r[:], start=True, stop=True
                    )
                    meang = pscr.tile([GPC, NCH, 1], F32, tag="st_mean")
                    ex2g = pscr.tile([GPC, NCH, 1], F32, tag="st_ex2")
                    nc.vector.tensor_scalar_mul(meang[:], psg[:, :, 0:1], 1.0 / 16.0)
                    nc.vector.tensor_scalar_mul(ex2g[:], psg[:, :, 1:2], 1.0 / 16.0)
                    varg = pscr.tile([GPC, NCH, 1], F32, tag="st_var")
                    nc.vector.tensor_mul(varg[:], meang[:], meang[:])
                    nc.vector.tensor_sub(varg[:], ex2g[:], varg[:])
                    stdg = pscr.tile([GPC, NCH, 1], F32, tag="st_std")
                    nc.scalar.activation(stdg[:], varg[:], ACTF.Sqrt, bias=eps_t[:])
                    rstdg = pscr.tile([GPC, NCH, 1], F32, tag="st_rstd")
                    nc.vector.reciprocal(rstdg[:], stdg[:])
                    st2 = pscr.tile([GPC, NCH, 2], F32R, tag="st2")
                    nc.vector.tensor_copy(st2[:, :, 0:1], rstdg[:])
                    nc.vector.tensor_copy(st2[:, :, 1:2], meang[:])
                    pse = psA.tile([128, NCH, 2], F32, tag="t", bufs=2)
                    nc.tensor.matmul(pse[:], e8[:], st2[:], start=True, stop=True)
                    # scale = rstd * gamma ; shift = beta - mean * scale
                    nc.vector.tensor_mul(scale4[:], pse[:, :, 0:1], gnw4[:])
                    tmp4 = pscr.tile([128, NCH], F32, tag="st_tmp")
                    nc.vector.tensor_mul(tmp4[:], pse[:, :, 1:2], scale4[:])
                    nc.vector.tensor_sub(shift4[:], gnb4[:], tmp4[:])

                    # ---- pass 2: GN apply + Q/K/V projections (fp8 DoubleRow;
                    # x comes straight from the resident SBUF copy) ----
                    for jb in range(NJB):
                        hn = pscr.tile([128, NCH, 512], FP8, tag="hn")
                        for ci in range(NCH):
                            nc.scalar.activation(
                                hn[:, ci, :],
                                x32[:, ci, 512 * jb : 512 * (jb + 1)],
                                ACTF.Identity,
                                scale=scale4[:, ci : ci + 1],
                                bias=shift4[:, ci : ci + 1],
                            )
                        # Q and K, channel-major, fp8 (1/sqrt(C) folded into
                        # the exp scale in phase B, so q/k stay ~N(0,1));
                        # Q drains on ACT, K on DVE to balance the queues
                        for co in range(NCH):
                            psq = psA.tile([128, 512], F32, tag="m", bufs=4)
                            for ph in range(2):
                                nc.tensor.matmul(
                                    psq[:],
                                    wqT[:, 2 * ph : 2 * ph + 2, 128 * co : 128 * (co + 1)],
                                    hn[:, 2 * ph : 2 * ph + 2, :],
                                    start=(ph == 0),
                                    stop=(ph == 1),
                                    perf_mode=DR,
                                )
                            nc.scalar.activation(
                                qt[:, co, 512 * jb : 512 * (jb + 1)],
                                psq[:],
                                ACTF.Identity,
                                scale=1.0 / 16.0,
                                bias=bq4[:, co : co + 1],
                            )
                        for co in range(NCH):
                            psk = psA.tile([128, 512], F32, tag="m", bufs=4)
                            for ph in range(2):
                                nc.tensor.matmul(
                                    psk[:],
                                    wkT[:, 2 * ph : 2 * ph + 2, 128 * co : 128 * (co + 1)],
                                    hn[:, 2 * ph : 2 * ph + 2, :],
                                    start=(ph == 0),
                                    stop=(ph == 1),
                                    perf_mode=DR,
                                )
                            nc.scalar.activation(
                                kt[:, co, 512 * jb : 512 * (jb + 1)],
                                psk[:],
                                ACTF.Identity,
                                scale=1.0 / 16.0,
                                bias=bk4[:, co : co + 1],
                            )
                        # V[j, c] per j-subtile, token-major
                        for jtl in range(4):
                            psv = psA.tile([128, 512], F32, tag="m", bufs=4)
                            for ph in range(2):
                                nc.tensor.matmul(
                                    psv[:],
                                    hn[:, 2 * ph : 2 * ph + 2, 128 * jtl : 128 * (jtl + 1)],
                                    wvT[:, 2 * ph : 2 * ph + 2, :],
                                    start=(ph == 0),
                                    stop=(ph == 1),
                                    perf_mode=DR,
                                )
                            nc.vector.scalar_tensor_tensor(
                                out=vt[:, 4 * jb + jtl, :],
                                in0=psv[:],
                                scalar=1.0 / 16.0,
                                in1=bv_bcast[:],
                                op0=OP.mult,
                                op1=OP.add,
                            )

        # ---- phase B: single pass over all q-blocks, full K/V resident ----
        with (
            tc.tile_pool(name="poolB", bufs=1) as pB,
            tc.tile_pool(name="psB", bufs=1, space="PSUM") as psB,
        ):
            pending = None

            def emit_epilogue(p):
                # deferred final projection + bias + residual for a prior
                # q-block; spliced into the next q-block's PE stream so it
                # fills the scores->exp->PV latency bubble
                e_qb, e_osb, e_rbc, e_xb = p
                outs = pB.tile([128, NCH, 512], F32, tag="outs", bufs=2)
                for co in range(NCH):
                    psf = psB.tile([128, 512], F32, tag="f", bufs=1)
                    for ph in range(2):
                        nc.tensor.matmul(
                            psf[:],
                            woT[:, 2 * ph : 2 * ph + 2, 128 * co : 128 * (co + 1)],
                            e_osb[:, 2 * ph : 2 * ph + 2, :],
                            start=(ph == 0),
                            stop=(ph == 1),
                            perf_mode=DR,
                        )
                    nc.vector.tensor_mul(outs[:, co, :], psf[:], e_rbc[:])
                    nc.vector.tensor_add(
                        outs[:, co, :], outs[:, co, :], e_xb[:, co, :]
                    )
                nc.sync.dma_start(
                    out=out_r[:, :, 512 * e_qb : 512 * (e_qb + 1)], in_=outs[:]
                )

            for qb in range(NQB):
                # residual + output bias staged early on DVE (off the ACT
                # queue, consumed by the deferred epilogue next q-block)
                xb = pB.tile([128, NCH, 512], BF16, tag="xb", bufs=2)
                for co in range(NCH):
                    nc.vector.tensor_scalar_add(
                        xb[:, co, :],
                        x32[:, co, 512 * qb : 512 * (qb + 1)],
                        bo4[:, co : co + 1],
                    )
                # den accumulated in f32 (DVE converts fp8 exp tiles on read);
                # stored as f32r so the ones-matmul can consume it directly;
                # kept as two half-sums (one DVE op per exp pair), summed on
                # the PE via two accumulating psd matmuls
                den = pB.tile([128, 2, 512], F32R, tag="den", bufs=2)
                pso = [
                    psB.tile([128, 512], F32, tag="o", bufs=4, name="pso")
                    for _ in range(NCH)
                ]
                # software-pipelined: scores/exp of pair t+1 are emitted
                # before PV of pair t, so the PE has work to chew while the
                # ACT exp latency drains (pss bufs=3 covers the lookahead)
                pt_tiles = []

                def emit_pair(t):
                    pt8 = pB.tile([128, 2, 512], FP8, tag="pt", bufs=3)
                    pt_tiles.append(pt8)
                    for sub in range(2):
                        jt = 2 * t + sub
                        pss = psB.tile([128, 512], F32, tag="s", bufs=2)
                        for ph in range(2):
                            nc.tensor.matmul(
                                pss[:],
                                kt[:, 2 * ph : 2 * ph + 2, 128 * jt : 128 * (jt + 1)],
                                qt[:, 2 * ph : 2 * ph + 2, 512 * qb : 512 * (qb + 1)],
                                start=(ph == 0),
                                stop=(ph == 1),
                                perf_mode=DR,
                            )
                        nc.scalar.activation(
                            pt8[:, sub, :], pss[:], ACTF.Exp,
                            scale=SCALE_Q, bias=expb_t[:],
                        )
                    if t == 0:
                        nc.vector.tensor_copy(den[:], pt8[:])
                    else:
                        nc.vector.tensor_add(den[:], den[:].bitcast(F32), pt8[:])

                def emit_pv(t):
                    pt8 = pt_tiles[t]
                    for cc in range(NCH):
                        nc.tensor.matmul(
                            pso[cc][:],
                            vt[:, 2 * t : 2 * t + 2, 128 * cc : 128 * (cc + 1)],
                            pt8[:],
                            start=(t == 0),
                            stop=(t == NJT // 2 - 1),
                            perf_mode=DR,
                        )

                emit_pair(0)
                if pending is not None:
                    emit_epilogue(pending)
                    pending = None
                for t in range(1, NJT // 2):
                    emit_pair(t)
                    emit_pv(t - 1)
                emit_pv(NJT // 2 - 1)
                # partition-sum of den, broadcast of 1/den
                psd = psB.tile([1, 512], F32, tag="f", bufs=1)
                nc.tensor.matmul(psd[:], ones128[:], den[:, 0, :], start=True, stop=False)
                nc.tensor.matmul(psd[:], ones128[:], den[:, 1, :], start=False, stop=True)
                # osb is drained at 1/32 and woT carries x16 -> psf is half
                # of wo@O, so fold the compensating x2 into the denominator
                dtot = pB.tile([1, 512], F32R, tag="dtot", bufs=2)
                nc.vector.tensor_scalar_mul(dtot[:], psd[:], 0.5)
                # psb gets its own PSUM bank: the reciprocal holds a long read
                # on it and must not gate the next epilogue's psf matmuls
                psb = psB.tile([128, 512], F32, tag="b", bufs=1)
                nc.tensor.matmul(psb[:], ones1[:], dtot[:], start=True, stop=True)
                rbc = pB.tile([128, 512], F32, tag="rbc", bufs=2)
                nc.vector.reciprocal(rbc[:], psb[:])
                # drain unnormalized O^T to SBUF (frees the pso banks);
                # split across ACT and DVE so neither queue stalls the next
                # q-block's exp/den chain
                osb = pB.tile([128, NCH, 512], FP8, tag="osb", bufs=2)
                for cc in range(NCH):
                    if cc % 2 == 0:
                        nc.scalar.activation(
                            osb[:, cc, :], pso[cc][:], ACTF.Identity,
                            scale=1.0 / 32.0,
                        )
                    else:
                        nc.vector.tensor_scalar_mul(
                            osb[:, cc, :], pso[cc][:], 1.0 / 32.0
                        )
                pending = (qb, osb, rbc, xb)
            if pending is not None:
                emit_epilogue(pending)
                pending = None
    return nc


# ---------------------------------------------------------------------------
# Walrus in this container rejects instructions carrying more than ~2
# sync-wait commands ("Too many sync wait commands").  Hoist excess on_wait
# entries onto nofuse NOPs placed immediately before the instruction on the
# same engine (engines issue in-order, so blocking on the NOP first is
# equivalent).
def split_sync_waits(nc, max_waits=1):
    n_split = 0
    for bb in nc.main_func.blocks:
        insts = bb.instructions
        out = []
        for inst in insts:
            si = inst.sync_info
            if si is not None and si.on_wait is not None and len(si.on_wait) > max_waits:
                waits = list(si.on_wait)
                keep = waits[-max_waits:]
                extra = waits[:-max_waits]
                for i in range(0, len(extra), max_waits):
                    chunk = extra[i : i + max_waits]
                    nop = mybir.InstNoOp(
                        name=f"{inst.name}-sw{i}",
                        sync_info=mybir.SyncInfo(on_wait=chunk, on_update=[]),
                        bass_nofuse=True,
                        engine=inst.engine,
                    )
                    out.append(nop)
                    n_split += 1
                inst.sync_info = mybir.SyncInfo(
                    on_wait=keep, on_update=list(si.on_update or [])
                )
            out.append(inst)
        bb.instructions = out
    return n_split


B, H, W = 8, 64, 64
HW = H * W
N_CORES = 8
_CACHE = {}


def _get_nc():
    if "nc" not in _CACHE:
        nc = bass.Bass()
        build(nc, HW=HW)
        split_sync_waits(nc)
        _CACHE["nc"] = nc
    return _CACHE["nc"]


def _in_maps(inputs):
    import numpy as np
    arrs = {k: np.ascontiguousarray(np.asarray(v, dtype=np.float32)) for k, v in inputs.items()}
    x = arrs.pop("x").reshape(B, C, HW)
    return [{"x": x[i], **arrs} for i in range(N_CORES)]


def kernel(**inputs):
    import numpy as np
    from concourse.bass_utils import run_bass_kernel_spmd

    nc = _get_nc()
    res = run_bass_kernel_spmd(nc, _in_maps(inputs), list(range(N_CORES)))
    out = np.stack([res.results[i]["out"] for i in range(N_CORES)])
    return out.reshape(B, C, H, W).astype(np.float32)


def kernel_traced(**inputs):
    """Like kernel() but with NTFF profiling; returns (output, BassKernelResults)."""
    import numpy as np
    from concourse.bass_utils import run_bass_kernel_spmd

    nc = _get_nc()
    res = run_bass_kernel_spmd(
        nc, _in_maps(inputs), list(range(N_CORES)), trace=True
    )
    out = np.stack([res.results[i]["out"] for i in range(N_CORES)])
    return out.reshape(B, C, H, W).astype(np.float32), res
